# revision 8
# baseline (speedup 1.0000x reference)
"""ClusterFeatureNet-with-attention Trainium2 kernel.

Shards the cluster axis C=512 across 8 NeuronCores (64 clusters/core/side).
Each core gathers its clusters' rows from a replicated, host-packed
[N, 260] table (256 feat cols + 3 pt cols + pad) via indirect DMA, and runs
attention + mean-pool + MLP locally.

Key algebraic simplification: the reference computes
    feat = mean_p( (softmax(QK^T/s) @ V) @ Wf + bf )
Mean-pooling commutes with the matmuls, so per cluster we only need
    w[q]  = (1/P) sum_p softmax(QK^T/s)[p, q]          (column mean of attn)
    m     = w @ V                                       ([1, H] row)
    feat  = m @ Wf + (bv @ Wf + bf)  -> MLP             (batched over clusters)
which removes the P×P×H and P×H×O per-point matmuls entirely.

Matmuls run in float32r (full-rate PE; fp32 runs at 1/4 rate). All PSUM
accumulation is fp32; operands are rounded to f32r at the producing
PSUM->SBUF copy, as the BIR verifier requires.
"""

import math
import sys

for _p in ("/opt/trn_rl_repo",):
    if _p not in sys.path:
        sys.path.insert(0, _p)

import numpy as np

import concourse.bass as bass
import concourse.mybir as mybir
import concourse.tile as tile
from concourse import bacc
from concourse.bass_utils import run_bass_kernel_spmd
from concourse.masks import make_identity

# Problem dims (hardcoded per spec)
N = 131072
C = 512
PC = 256          # points per cluster
D = 256           # feature dim
H = 512           # attention hidden dim
O = 256           # output dim
NCORES = 8
CPC = C // NCORES  # clusters per core per side = 64
P = 128            # partitions
TW = 260           # packed table width: D feats + 3 coords + 1 pad
F32 = mybir.dt.float32
I32 = mybir.dt.int32

USE_F32R = True    # run big matmuls in float32r (4x faster PE, slightly reduced precision)

SIDES = ("src", "tgt")


def build_program(cpc=CPC, enable_asserts=False):
    """Build + compile the per-core SPMD program."""
    FR = mybir.dt.float32r if USE_F32R else F32
    nc = bacc.Bacc(
        "TRN2",
        target_bir_lowering=False,
        debug=False,
        enable_asserts=enable_asserts,
        num_devices=NCORES,
    )

    # ---- DRAM I/O ----
    tbl = {s: nc.dram_tensor(f"tbl_{s}", [N, TW], F32, kind="ExternalInput").ap()
           for s in SIDES}
    idxT = {s: nc.dram_tensor(f"idxT_{s}", [P, 2, cpc], I32, kind="ExternalInput").ap()
            for s in SIDES}
    Wq_d = nc.dram_tensor("Wq", [D, H], F32, kind="ExternalInput").ap()   # pre-scaled 1/sqrt(H)
    Wk_d = nc.dram_tensor("Wk", [D, H], F32, kind="ExternalInput").ap()
    Wv_d = nc.dram_tensor("Wv", [D, H], F32, kind="ExternalInput").ap()
    Wf_d = nc.dram_tensor("Wf", [H, O], F32, kind="ExternalInput").ap()
    W1_d = nc.dram_tensor("W1", [O, H], F32, kind="ExternalInput").ap()
    W2_d = nc.dram_tensor("W2", [H, O], F32, kind="ExternalInput").ap()
    bq_d = nc.dram_tensor("bq", [P, 4], F32, kind="ExternalInput").ap()   # pre-scaled, [128,4]
    bk_d = nc.dram_tensor("bk", [P, 4], F32, kind="ExternalInput").ap()
    bvT_d = nc.dram_tensor("bvT", [P, 4], F32, kind="ExternalInput").ap()
    bf_d = nc.dram_tensor("bf", [1, O], F32, kind="ExternalInput").ap()
    b1_d = nc.dram_tensor("b1", [1, H], F32, kind="ExternalInput").ap()
    b2_d = nc.dram_tensor("b2", [1, O], F32, kind="ExternalInput").ap()

    feat_o = {s: nc.dram_tensor(f"feat_{s}_o", [cpc, O], F32, kind="ExternalOutput").ap()
              for s in SIDES}
    coord_o = {s: nc.dram_tensor(f"coord_{s}_o", [cpc, 3], F32, kind="ExternalOutput").ap()
               for s in SIDES}

    with tile.TileContext(nc) as tc:
        with (
            tc.tile_pool(name="wp", bufs=1) as wp,
            tc.tile_pool(name="sp", bufs=2) as sp,
            tc.tile_pool(name="xp", bufs=2) as xp,
            tc.tile_pool(name="mp", bufs=1) as mp,
            tc.tile_pool(name="fp", bufs=2) as fp,
            tc.tile_pool(name="pp_mm", bufs=5, space="PSUM") as pp_mm,
            tc.tile_pool(name="pp_sm", bufs=3, space="PSUM") as pp_sm,
        ):
            # ---- static weights: DMA fp32 -> staging, DVE-round -> f32r ----
            def load_r(dram_ap, shape, name):
                stg = sp.tile(list(shape), F32, tag="stg", name=f"{name}_stg")
                nc.sync.dma_start(stg[:], dram_ap)
                t = wp.tile(list(shape), FR, name=f"{name}_r")
                nc.vector.tensor_copy(t[:], stg[:])
                return t

            Wq_sb = load_r(Wq_d.rearrange("(dj p) h -> p dj h", p=P), [P, 2, H], "Wq")
            Wk_sb = load_r(Wk_d.rearrange("(dj p) h -> p dj h", p=P), [P, 2, H], "Wk")
            Wv_sb = load_r(Wv_d.rearrange("(dj p) h -> p dj h", p=P), [P, 2, H], "Wv")
            Wf_sb = load_r(Wf_d.rearrange("(hj p) o -> p hj o", p=P), [P, 4, O], "Wf")
            W1_sb = load_r(W1_d.rearrange("(oj p) h -> p oj h", p=P), [P, 2, H], "W1")
            W2_sb = load_r(W2_d.rearrange("(hj p) o -> p hj o", p=P), [P, 4, O], "W2")
            bvT_sb = load_r(bvT_d, [P, 4], "bvT")
            bf_sb = load_r(bf_d, [1, O], "bf")
            b1_sb = load_r(b1_d, [1, H], "b1")
            b2_sb = load_r(b2_d, [1, O], "b2")

            # fp32 per-partition bias vectors (DVE scalar operands, not matmul inputs)
            bq_sb = wp.tile([P, 4], F32)
            nc.sync.dma_start(bq_sb[:], bq_d)
            bk_sb = wp.tile([P, 4], F32)
            nc.sync.dma_start(bk_sb[:], bk_d)
            idxT_sb = {}
            for s in SIDES:
                idxT_sb[s] = wp.tile([P, 2, cpc], I32, name=f"idxT_sb_{s}")
                nc.sync.dma_start(idxT_sb[s][:], idxT[s])

            ident = wp.tile([P, P], F32)
            make_identity(nc, ident[:])
            invPCc = wp.tile([P, 1], F32)
            nc.vector.memset(invPCc[:], 1.0 / PC)

            cstg = sp.tile([1, cpc], F32, tag="cstg")
            nc.vector.memset(cstg[:], 1.0)
            ones64 = wp.tile([1, cpc], FR)
            nc.vector.tensor_copy(ones64[:], cstg[:])
            one11 = wp.tile([1, 1], FR)
            nc.vector.tensor_copy(one11[:], cstg[:, :1])
            invP = wp.tile([1, 1], F32)
            nc.vector.memset(invP[:], 1.0 / PC)

            # bfp = bv @ Wf + bf  (folds the V bias through the mean-pool)
            bfp_ps = pp_sm.tile([1, O], F32, tag="small")
            for hj in range(4):
                nc.tensor.matmul(
                    out=bfp_ps[:],
                    lhsT=bvT_sb[:, hj:hj + 1],
                    rhs=Wf_sb[:, hj, :],
                    start=(hj == 0),
                    stop=False,
                )
            nc.tensor.matmul(
                out=bfp_ps[:], lhsT=one11[:], rhs=bf_sb[:],
                start=False, stop=True,
            )
            bfp_sb = wp.tile([1, O], FR)
            nc.vector.tensor_copy(bfp_sb[:], bfp_ps[:])

            for s in SIDES:
                Mall = mp.tile([cpc, H], F32, name=f"Mall_{s}", tag=f"Mall_{s}")

                for c in range(cpc):
                    # -- gather this cluster's 256 rows (feats + pts packed) --
                    xg = xp.tile([P, 2, TW], F32, tag="xg", bufs=4)
                    for pj in range(2):
                        nc.gpsimd.indirect_dma_start(
                            out=xg[:, pj, :],
                            out_offset=None,
                            in_=tbl[s][:],
                            in_offset=bass.IndirectOffsetOnAxis(
                                ap=idxT_sb[s][:, pj, c:c + 1], axis=0),
                        )

                    # -- coords: mean of gathered pts rows (plain fp32) --
                    cd_ps = pp_sm.tile([1, 4], F32, tag="small", name="cd_ps")
                    for pj in range(2):
                        nc.tensor.matmul(
                            out=cd_ps[:, :3],
                            lhsT=invPCc[:],
                            rhs=xg[:, pj, D:D + 3],
                            start=(pj == 0),
                            stop=(pj == 1),
                        )
                    cd_sb = xp.tile([1, 4], F32, tag="cd")
                    nc.vector.tensor_copy(cd_sb[:, :3], cd_ps[:, :3])
                    nc.sync.dma_start(out=coord_o[s][c:c + 1, :], in_=cd_sb[:, :3])

                    # -- transpose x -> xT [D, P] --
                    xt_ps = pp_mm.tile([P, 4, P], F32, tag="mm", name="xt_ps")
                    for dj in range(2):
                        for pj in range(2):
                            nc.tensor.transpose(
                                out=xt_ps[:, dj * 2 + pj, :],
                                in_=xg[:, pj, dj * P:(dj + 1) * P],
                                identity=ident[:],
                            )
                    xT_sb = xp.tile([P, 2, 2 * P], FR, tag="xT")
                    for dj in range(2):
                        nc.vector.tensor_copy(
                            xT_sb[:, dj, :], xt_ps[:, dj * 2:dj * 2 + 2, :])

                    # -- QT = (x Wq)^T (+bq, pre-scaled 1/sqrt(H)); KT likewise --
                    qt_ps = [pp_mm.tile([P, 2, PC], F32, tag="mm", name=f"qt_ps{t}")
                             for t in range(2)]
                    kt_ps = [pp_mm.tile([P, 2, PC], F32, tag="mm", name=f"kt_ps{t}")
                             for t in range(2)]
                    for W_sb, ps in ((Wq_sb, qt_ps), (Wk_sb, kt_ps)):
                        for hj in range(4):
                            for dj in range(2):
                                nc.tensor.matmul(
                                    out=ps[hj // 2][:, hj % 2, :],
                                    lhsT=W_sb[:, dj, hj * P:(hj + 1) * P],
                                    rhs=xT_sb[:, dj, :],
                                    start=(dj == 0),
                                    stop=(dj == 1),
                                )
                    QT_sb = xp.tile([P, 4, PC], FR, tag="QT")
                    KT_sb = xp.tile([P, 4, PC], FR, tag="KT")
                    for hj in range(4):
                        nc.vector.tensor_scalar_add(
                            out=QT_sb[:, hj, :],
                            in0=qt_ps[hj // 2][:, hj % 2, :],
                            scalar1=bq_sb[:, hj:hj + 1],
                        )
                        nc.vector.tensor_scalar_add(
                            out=KT_sb[:, hj, :],
                            in0=kt_ps[hj // 2][:, hj % 2, :],
                            scalar1=bk_sb[:, hj:hj + 1],
                        )

                    # -- V = x Wv  [P, H] (bias folded into bfp) --
                    v_ps = [pp_mm.tile([P, H], F32, tag="mm", name=f"v_ps{t}")
                            for t in range(2)]
                    for pj in range(2):
                        for dj in range(2):
                            nc.tensor.matmul(
                                out=v_ps[pj][:],
                                lhsT=xT_sb[:, dj, pj * P:(pj + 1) * P],
                                rhs=Wv_sb[:, dj, :],
                                start=(dj == 0),
                                stop=(dj == 1),
                            )
                    V_sb = xp.tile([P, 2, H], FR, tag="V")
                    for pj in range(2):
                        nc.scalar.copy(out=V_sb[:, pj, :], in_=v_ps[pj][:])

                    # -- S = Q' K'^T  [p, q] --
                    s_ps = pp_mm.tile([P, 2, PC], F32, tag="mm", name="s_ps")
                    for pj in range(2):
                        for hj in range(4):
                            nc.tensor.matmul(
                                out=s_ps[:, pj, :],
                                lhsT=QT_sb[:, hj, pj * P:(pj + 1) * P],
                                rhs=KT_sb[:, hj, :],
                                start=(hj == 0),
                                stop=(hj == 3),
                            )

                    # -- softmax pieces: E = exp(S), rowsums, reciprocal --
                    # (scores are O(0.5) here so no max-subtraction is needed)
                    E_sb = xp.tile([P, 2, PC], FR, tag="E")
                    rs = xp.tile([P, 2], F32, tag="rs")
                    for pj in range(2):
                        nc.scalar.activation(
                            out=E_sb[:, pj, :],
                            in_=s_ps[:, pj, :],
                            func=mybir.ActivationFunctionType.Exp,
                            accum_out=rs[:, pj:pj + 1],
                        )
                    ri = xp.tile([P, 2], F32, tag="ri")
                    nc.vector.reciprocal(ri[:], rs[:])
                    ri_r = xp.tile([P, 2], FR, tag="ri_r")
                    nc.vector.tensor_copy(ri_r[:], ri[:])

                    # -- w_raw[q] = sum_p E[p,q]/rowsum[p]; then wT = w_raw/P --
                    w_ps = pp_sm.tile([1, PC], F32, tag="small", name="w_ps")
                    for pj in range(2):
                        nc.tensor.matmul(
                            out=w_ps[:],
                            lhsT=ri_r[:, pj:pj + 1],
                            rhs=E_sb[:, pj, :],
                            start=(pj == 0),
                            stop=(pj == 1),
                        )
                    w_sb = xp.tile([1, PC], F32, tag="w")
                    nc.vector.tensor_copy(w_sb[:], w_ps[:])
                    wT_ps = pp_sm.tile([P, 2], F32, tag="small", name="wT_ps")
                    for qj in range(2):
                        nc.tensor.matmul(
                            out=wT_ps[:, qj:qj + 1],
                            lhsT=w_sb[:, qj * P:(qj + 1) * P],
                            rhs=invP[:],
                            start=True,
                            stop=True,
                        )
                    wT_sb = xp.tile([P, 2], FR, tag="wT")
                    nc.vector.tensor_copy(wT_sb[:], wT_ps[:])

                    # -- m = w @ V  [1, H]; stash into Mall[c] --
                    m_ps = pp_sm.tile([1, H], F32, tag="small", name="m_ps")
                    for qj in range(2):
                        nc.tensor.matmul(
                            out=m_ps[:],
                            lhsT=wT_sb[:, qj:qj + 1],
                            rhs=V_sb[:, qj, :],
                            start=(qj == 0),
                            stop=(qj == 1),
                        )
                    m_sb = xp.tile([1, H], F32, tag="m")
                    nc.vector.tensor_copy(m_sb[:], m_ps[:])
                    nc.sync.dma_start(out=Mall[c:c + 1, :], in_=m_sb[:])

                # ---- batched tail for this side: feat = MLP(Mall @ Wf + bfp) ----
                mt_ps = pp_mm.tile([P, 4, cpc], F32, tag="mm", name="mt_ps")
                for hj in range(4):
                    nc.tensor.transpose(
                        out=mt_ps[:, hj, :],
                        in_=Mall[:, hj * P:(hj + 1) * P],
                        identity=ident[:cpc, :cpc],
                    )
                MT_sb = fp.tile([P, 4, cpc], FR, tag="MT")
                nc.vector.tensor_copy(MT_sb[:], mt_ps[:])

                ft_ps = pp_sm.tile([cpc, O], F32, tag="small", name="ft_ps")
                for hj in range(4):
                    nc.tensor.matmul(
                        out=ft_ps[:], lhsT=MT_sb[:, hj, :], rhs=Wf_sb[:, hj, :],
                        start=(hj == 0), stop=False,
                    )
                nc.tensor.matmul(
                    out=ft_ps[:], lhsT=ones64[:], rhs=bfp_sb[:],
                    start=False, stop=True,
                )
                feat_sb = fp.tile([cpc, O], F32, tag="feat")
                nc.vector.tensor_copy(feat_sb[:], ft_ps[:])

                fT_ps = pp_sm.tile([P, 2, cpc], F32, tag="small", name="fT_ps")
                for oj in range(2):
                    nc.tensor.transpose(
                        out=fT_ps[:, oj, :],
                        in_=feat_sb[:, oj * P:(oj + 1) * P],
                        identity=ident[:cpc, :cpc],
                    )
                fT_sb = fp.tile([P, 2, cpc], FR, tag="fT")
                nc.vector.tensor_copy(fT_sb[:], fT_ps[:])

                h_ps = pp_sm.tile([cpc, H], F32, tag="small", name="h_ps")
                for oj in range(2):
                    nc.tensor.matmul(
                        out=h_ps[:], lhsT=fT_sb[:, oj, :], rhs=W1_sb[:, oj, :],
                        start=(oj == 0), stop=False,
                    )
                nc.tensor.matmul(
                    out=h_ps[:], lhsT=ones64[:], rhs=b1_sb[:],
                    start=False, stop=True,
                )
                hid_sb = fp.tile([cpc, H], F32, tag="hid")
                nc.scalar.activation(
                    out=hid_sb[:], in_=h_ps[:],
                    func=mybir.ActivationFunctionType.Relu,
                )

                hT_ps = pp_sm.tile([P, 4, cpc], F32, tag="small", name="hT_ps")
                for hj in range(4):
                    nc.tensor.transpose(
                        out=hT_ps[:, hj, :],
                        in_=hid_sb[:, hj * P:(hj + 1) * P],
                        identity=ident[:cpc, :cpc],
                    )
                hT_sb = fp.tile([P, 4, cpc], FR, tag="hT")
                nc.vector.tensor_copy(hT_sb[:], hT_ps[:])

                o_ps = pp_sm.tile([cpc, O], F32, tag="small", name="o_ps")
                for hj in range(4):
                    nc.tensor.matmul(
                        out=o_ps[:], lhsT=hT_sb[:, hj, :], rhs=W2_sb[:, hj, :],
                        start=(hj == 0), stop=False,
                    )
                nc.tensor.matmul(
                    out=o_ps[:], lhsT=ones64[:], rhs=b2_sb[:],
                    start=False, stop=True,
                )
                out_sb = fp.tile([cpc, O], F32, tag="out")
                nc.vector.tensor_copy(out_sb[:], o_ps[:])
                nc.sync.dma_start(out=feat_o[s][:], in_=out_sb[:])

    nc.compile()
    return nc


def prep_inputs(src_feats, tgt_feats, src_pts, tgt_pts,
                idx_spt2pts_src, idx_spt2pts_tgt,
                Wq, bq, Wk, bk, Wv, bv, Wf, bf, W1, b1, W2, b2, cpc=CPC):
    """Host-side packing. Returns list of 8 per-core input dicts."""
    f32 = np.float32
    sf = np.asarray(src_feats, f32)[0]
    tf = np.asarray(tgt_feats, f32)[0]
    sp = np.asarray(src_pts, f32)[0]
    tp = np.asarray(tgt_pts, f32)[0]
    tbl = {}
    for s, feats, pts in (("src", sf, sp), ("tgt", tf, tp)):
        t = np.zeros((N, TW), f32)
        t[:, :D] = feats
        t[:, D:D + 3] = pts
        tbl[s] = t
    idx = {
        "src": np.ascontiguousarray(np.asarray(idx_spt2pts_src).astype(np.int32)),
        "tgt": np.ascontiguousarray(np.asarray(idx_spt2pts_tgt).astype(np.int32)),
    }
    scale = 1.0 / math.sqrt(float(H))
    Wq = np.asarray(Wq, f32) * scale
    bq = np.asarray(bq, f32) * scale
    shared = {
        "Wq": np.ascontiguousarray(Wq),
        "Wk": np.ascontiguousarray(np.asarray(Wk, f32)),
        "Wv": np.ascontiguousarray(np.asarray(Wv, f32)),
        "Wf": np.ascontiguousarray(np.asarray(Wf, f32)),
        "W1": np.ascontiguousarray(np.asarray(W1, f32)),
        "W2": np.ascontiguousarray(np.asarray(W2, f32)),
        "bq": np.ascontiguousarray(bq.reshape(4, P).T),
        "bk": np.ascontiguousarray(np.asarray(bk, f32).reshape(4, P).T),
        "bvT": np.ascontiguousarray(np.asarray(bv, f32).reshape(4, P).T),
        "bf": np.ascontiguousarray(np.asarray(bf, f32).reshape(1, O)),
        "b1": np.ascontiguousarray(np.asarray(b1, f32).reshape(1, H)),
        "b2": np.ascontiguousarray(np.asarray(b2, f32).reshape(1, O)),
        "tbl_src": tbl["src"],
        "tbl_tgt": tbl["tgt"],
    }
    in_maps = []
    for k in range(NCORES):
        m = dict(shared)
        for s in SIDES:
            sl = idx[s][k * cpc:(k + 1) * cpc]              # [cpc, 256]
            idxt = sl.reshape(cpc, 2, P).transpose(2, 1, 0)  # [128, 2, cpc]
            m[f"idxT_{s}"] = np.ascontiguousarray(idxt)
        in_maps.append(m)
    return in_maps


_PROG = None
last_result = None  # BassKernelResults of the most recent run (for test harnesses)


def _get_prog():
    global _PROG
    if _PROG is None:
        _PROG = build_program()
    return _PROG


def kernel(**inputs):
    global last_result
    nc = _get_prog()
    in_maps = prep_inputs(**inputs)
    res = run_bass_kernel_spmd(nc, in_maps, core_ids=list(range(NCORES)))
    last_result = res
    results = res.results
    feat_src = np.concatenate([results[k]["feat_src_o"] for k in range(NCORES)], axis=0)
    feat_tgt = np.concatenate([results[k]["feat_tgt_o"] for k in range(NCORES)], axis=0)
    coord_src = np.concatenate([results[k]["coord_src_o"] for k in range(NCORES)], axis=0)
    coord_tgt = np.concatenate([results[k]["coord_tgt_o"] for k in range(NCORES)], axis=0)
    return feat_src, feat_tgt, coord_src, coord_tgt
